# revision 4
# baseline (speedup 1.0000x reference)
"""Trainium2 Bass kernel for nn_CutLayer (histogram_binning) — v2.

Strategy (data-parallel over events, 8 cores):
  L1 minmax: chunked DMA of the fp32 feature column + DVE min/max reduces.
  L2 counts in u-space: host preps u16 = fp16((x-gmin)*inv_h) and a
      signal-masked copy us16 (sentinel 60000). Edges are the integers
      0..50, compile-time immediates. Per-edge cumulative counts are
      computed as fused compare+accumulate passes split across three
      engines:
        - DVE: tensor_scalar(is_le)+accum on fp16 (4x perf mode),
        - ACT: Sign activation + accum,
        - GPSIMD: tensor_scalar(is_le)+accum.
      Host corrects counts exactly using a near-edge candidate set
      (fp16 rounding can only flip compares within |u - j| < W), then
      replicates the reference's tiny E^2 pair search bit-exactly with
      eager CPU jax.
  L3 pred: case-specialized fp16 compare(s) on the u16 stream, chunked
      and pipelined; host overwrites near-edge candidates with the exact
      fp32 predicate.

Events per core: 1_000_000; device handles 128*7812 = 999_936; the
64-per-core remainder is handled exactly on the host.
"""

from contextlib import ExitStack

import numpy as np

import concourse.bass as bass
import concourse.mybir as mybir
from concourse.bass_utils import run_bass_kernel_spmd

N = 8_000_000
N_CORES = 8
CORE_N = N // N_CORES            # 1_000_000
P = 128
F = 7812                         # free-dim columns per partition
H = F // 2
Q = F // 4                       # 1953
DEV_N = P * F                    # 999_936 device events per core
N_DEV_TOT = DEV_N * N_CORES      # 7_999_488
N_BINS = 50
E = N_BINS + 1                   # 51 edges
EPS = 1e-7
SENT = np.float16(60000.0)       # sentinel > any u value (u <= ~51)
W_U = 0.05                       # candidate window in u units

# Streams are class-compacted: S = u16 of signal events (dense), B = u16 of
# background events (dense), each padded with SENT to [P, F2].
F2 = 3936                        # 128*3936 = 503_808 slots (+7.7 sigma binomial)
STREAM_N = P * F2

# stat list: (edge j, region r) with r=0 -> S, r=1 -> B; edges 0 and 50 are
# derived exactly on the host (x<=gmin ties and x<=gmax trivially), so only
# edges 1..49 need device stats: 98 total.
# Region-major so the DVE (which gets S stats) can start as soon as S lands.
# First N_VST stats go to DVE (STT+ones, is_le), rest to ACT (Sign).
DEV_EDGES = list(range(1, N_BINS))
ALL_STATS = [(j, 0) for j in DEV_EDGES] + [(j, 1) for j in DEV_EDGES]
N_VST = 46                       # DVE stats (tuned: 4.21us/stat vs ACT 3.68)
VST = ALL_STATS[:N_VST]
SST = ALL_STATS[N_VST:]
N_SST = len(SST)

FP32 = mybir.dt.float32
FP16 = mybir.dt.float16
I32 = mybir.dt.int32
AX = mybir.AxisListType
OP = mybir.AluOpType
ACT = mybir.ActivationFunctionType

CORE_IDS = list(range(N_CORES))


# --------------------------------------------------------------------------
# Bass programs
# --------------------------------------------------------------------------

MM_CH = 8                        # minmax DMA/compute chunks
MM_B = [round(F * c / MM_CH) for c in range(MM_CH + 1)]  # chunk boundaries


def _build_minmax():
    nc = bass.Bass()
    x = nc.declare_dram_parameter("x", [DEV_N], FP32, isOutput=False)
    mm = nc.declare_dram_parameter("mm", [P, 2 * MM_CH], FP32, isOutput=True)
    with ExitStack() as es:
        ec = es.enter_context
        xt = ec(nc.sbuf_tensor([P, F], FP32))
        acc = ec(nc.sbuf_tensor([P, 2 * MM_CH], FP32))
        ds = [ec(nc.semaphore(f"d{c}")) for c in range(MM_CH)]
        vsem = ec(nc.semaphore("vsem"))
        dso = ec(nc.semaphore("dso"))
        block = ec(nc.Block())

        @block.sync
        def _(sync):
            xv = x[:].rearrange("(p f) -> p f", p=P)
            for c in range(MM_CH):
                sync.dma_start(
                    xt[:, MM_B[c]:MM_B[c+1]], xv[:, MM_B[c]:MM_B[c+1]]
                ).then_inc(ds[c], 16)
            sync.wait_ge(vsem, 2 * MM_CH)
            sync.dma_start(mm[:], acc[:]).then_inc(dso, 16)
            sync.wait_ge(dso, 16)

        @block.vector
        def _(vector):
            for c in range(MM_CH):
                vector.wait_ge(ds[c], 16)
                sl = xt[:, MM_B[c]:MM_B[c+1]]
                vector.tensor_reduce(acc[:, c:c+1], sl, axis=AX.X, op=OP.min).then_inc(vsem, 1)
                vector.tensor_reduce(acc[:, MM_CH+c:MM_CH+c+1], sl, axis=AX.X, op=OP.max).then_inc(vsem, 1)
    return nc


def _build_counts():
    nc = bass.Bass()
    sd = nc.declare_dram_parameter("sd", [STREAM_N], FP16, isOutput=False)
    bd = nc.declare_dram_parameter("bd", [STREAM_N], FP16, isOutput=False)
    ned = nc.declare_dram_parameter("ned", [P, E], FP32, isOutput=False)
    # one extra accumulator column: stat 0 runs as two half-stream instrs
    # (cols 0 and N_VST) so the DVE starts before the S stream fully lands
    av = nc.declare_dram_parameter("acc_v", [P, N_VST + 1], FP32, isOutput=True)
    asn = nc.declare_dram_parameter("acc_s", [P, N_SST], FP32, isOutput=True)
    H2 = F2 // 2
    with ExitStack() as es:
        ec = es.enter_context
        st = ec(nc.sbuf_tensor([P, F2], FP16))
        bt = ec(nc.sbuf_tensor([P, F2], FP16))
        ones = ec(nc.sbuf_tensor([P, F2], FP16))
        scrv = ec(nc.sbuf_tensor([P, F2], FP16))
        scrs = ec(nc.sbuf_tensor([P, F2], FP16))
        edt = ec(nc.sbuf_tensor([P, E], FP32))
        atv = ec(nc.sbuf_tensor([P, N_VST + 1], FP32))
        ats = ec(nc.sbuf_tensor([P, N_SST], FP32))
        dss0 = ec(nc.semaphore("dss0"))
        dss = ec(nc.semaphore("dss"))
        dsb = ec(nc.semaphore("dsb"))
        dne = ec(nc.semaphore("dne"))
        vsem = ec(nc.semaphore("vsem"))
        ssem = ec(nc.semaphore("ssem"))
        dso = ec(nc.semaphore("dso"))
        block = ec(nc.Block())

        @block.sync
        def _(sync):
            sv = sd[:].rearrange("(p f) -> p f", p=P)
            bv = bd[:].rearrange("(p f) -> p f", p=P)
            sync.dma_start(edt[:], ned[:]).then_inc(dne, 16)
            sync.dma_start(st[:, 0:H2], sv[:, 0:H2]).then_inc(dss0, 16)
            sync.dma_start(st[:, H2:F2], sv[:, H2:F2]).then_inc(dss, 16)
            sync.dma_start(bt[:], bv).then_inc(dsb, 16)
            sync.wait_ge(ssem, N_SST)
            sync.dma_start(asn[:], ats[:]).then_inc(dso, 16)
            sync.wait_ge(vsem, N_VST + 1)
            sync.dma_start(av[:], atv[:]).then_inc(dso, 16)
            sync.wait_ge(dso, 32)

        # DVE: STT is_le * ones with fp32 accum; stats in VST order (S first).
        # Stat 0 runs as two half-stream instrs so compute starts once the
        # first half of S has landed.
        @block.vector
        def _(vector):
            vector.memset(ones[:], 1.0)
            j0, _r0 = VST[0]
            vector.wait_ge(dss0, 16)
            vector.scalar_tensor_tensor(
                scrv[:, 0:H2], st[:, 0:H2], float(j0), ones[:, 0:H2],
                op0=OP.is_le, op1=OP.mult,
                accum_out=atv[:, 0:1],
            ).then_inc(vsem, 1)
            vector.wait_ge(dss, 16)
            vector.scalar_tensor_tensor(
                scrv[:, 0:H2], st[:, H2:F2], float(j0), ones[:, 0:H2],
                op0=OP.is_le, op1=OP.mult,
                accum_out=atv[:, N_VST:N_VST + 1],
            ).then_inc(vsem, 1)
            waited = {0}
            for i, (j, r) in enumerate(VST):
                if i == 0:
                    continue
                sem = dss if r == 0 else dsb
                if r not in waited:
                    waited.add(r)
                    vector.wait_ge(sem, 16)
                src = st if r == 0 else bt
                vector.scalar_tensor_tensor(
                    scrv[:], src[:], float(j), ones[:],
                    op0=OP.is_le, op1=OP.mult,
                    accum_out=atv[:, i:i + 1],
                ).then_inc(vsem, 1)

        # ACT: Sign with fp32 accum; bias = -j from the edges tile.
        @block.scalar
        def _(scalar):
            scalar.wait_ge(dne, 16)
            # dummy pass on the (tiny, already-loaded) edges tile: pulls the
            # Sign ACT table load off the critical path while streams land
            scalar.activation(ats[:, 0:1], edt[:, 0:1], ACT.Sign,
                              bias=edt[:, 0:1], scale=1.0)
            waited = set()
            for i, (j, r) in enumerate(SST):
                if r not in waited:
                    waited.add(r)
                    if r == 0:
                        scalar.wait_ge(dss0, 16)
                        scalar.wait_ge(dss, 16)
                    else:
                        scalar.wait_ge(dsb, 16)
                src = st if r == 0 else bt
                scalar.activation(
                    scrs[:], src[:], ACT.Sign,
                    bias=edt[:, j:j + 1], scale=1.0,
                    accum_out=ats[:, i:i + 1],
                ).then_inc(ssem, 1)
    return nc


PR_CH = 4                        # pred DMA/compute chunks
PR_B = [0, 2344, 4688, 6800, 7812]   # shrinking chunks: smaller final tail
PR_Q = max(PR_B[c+1] - PR_B[c] for c in range(PR_CH))


def _build_pred(case: int):
    """u-space predicate, chunked.
    0: u <= lo ; 1: u >= lo ; 2: (u >= lo) & (u <= up) ; 3: (u <= lo) + (u >= up)
    """
    nc = bass.Bass()
    u = nc.declare_dram_parameter("u", [DEV_N], FP16, isOutput=False)
    pr = nc.declare_dram_parameter("prm", [P, 8], FP32, isOutput=False)
    out = nc.declare_dram_parameter("pred", [DEV_N], FP16, isOutput=True)
    with ExitStack() as es:
        ec = es.enter_context
        ut = ec(nc.sbuf_tensor([P, F], FP16))
        po = ec(nc.sbuf_tensor([P, F], FP16))
        t1 = ec(nc.sbuf_tensor([P, PR_Q], FP16))
        t2 = ec(nc.sbuf_tensor([P, PR_Q], FP16))
        prm = ec(nc.sbuf_tensor([P, 8], FP32))
        dp = ec(nc.semaphore("dp"))
        ds = [ec(nc.semaphore(f"d{c}")) for c in range(PR_CH)]
        vsem = ec(nc.semaphore("vsem"))
        dso = ec(nc.semaphore("dso"))
        block = ec(nc.Block())

        @block.sync
        def _(sync):
            uv = u[:].rearrange("(p f) -> p f", p=P)
            ov = out[:].rearrange("(p f) -> p f", p=P)
            sync.dma_start(prm[:], pr[:]).then_inc(dp, 16)
            for c in range(PR_CH):
                sync.dma_start(
                    ut[:, PR_B[c]:PR_B[c+1]], uv[:, PR_B[c]:PR_B[c+1]]
                ).then_inc(ds[c], 16)
            for c in range(PR_CH):
                sync.wait_ge(vsem, c + 1)
                sync.dma_start(
                    ov[:, PR_B[c]:PR_B[c+1]], po[:, PR_B[c]:PR_B[c+1]]
                ).then_inc(dso, 16)
            sync.wait_ge(dso, 16 * PR_CH)

        @block.vector
        def _(vector):
            vector.wait_ge(dp, 16)
            lo = prm[:, 0:1]
            up = prm[:, 1:2]
            for c in range(PR_CH):
                vector.wait_ge(ds[c], 16)
                w = PR_B[c+1] - PR_B[c]
                uc = ut[:, PR_B[c]:PR_B[c+1]]
                oc = po[:, PR_B[c]:PR_B[c+1]]
                if case == 0:
                    vector.tensor_scalar(oc, uc, lo, 1.0, OP.is_le, OP.mult).then_inc(vsem, 1)
                elif case == 1:
                    vector.tensor_scalar(oc, uc, lo, 1.0, OP.is_ge, OP.mult).then_inc(vsem, 1)
                elif case == 2:
                    vector.tensor_scalar(t1[:, 0:w], uc, lo, 1.0, OP.is_ge, OP.mult)
                    vector.tensor_scalar(t2[:, 0:w], uc, up, 1.0, OP.is_le, OP.mult)
                    vector.tensor_tensor(oc, t1[:, 0:w], t2[:, 0:w], op=OP.mult).then_inc(vsem, 1)
                else:
                    vector.tensor_scalar(t1[:, 0:w], uc, lo, 1.0, OP.is_le, OP.mult)
                    vector.tensor_scalar(t2[:, 0:w], uc, up, 1.0, OP.is_ge, OP.mult)
                    vector.tensor_tensor(oc, t1[:, 0:w], t2[:, 0:w], op=OP.add).then_inc(vsem, 1)
    return nc


_PROGRAMS: dict = {}


def _prog(name):
    if name not in _PROGRAMS:
        if name.startswith("pred"):
            _PROGRAMS[name] = _build_pred(int(name[4:]))
        else:
            _PROGRAMS[name] = {
                "minmax": _build_minmax,
                "counts": _build_counts,
            }[name]()
    return _PROGRAMS[name]


# --------------------------------------------------------------------------
# Host orchestration
# --------------------------------------------------------------------------

LAST_EXEC_NS: list = []
_CACHE_SET = False


def _enable_jit_cache():
    global _CACHE_SET
    if _CACHE_SET:
        return
    _CACHE_SET = True
    try:
        import jax

        jax.config.update("jax_compilation_cache_dir", "/tmp/jax_bass_cache")
        jax.config.update("jax_persistent_cache_min_compile_time_secs", 1.0)
        jax.config.update("jax_persistent_cache_min_entry_size_bytes", 0)
    except Exception:
        pass


def _run(name, in_maps):
    import os

    _enable_jit_cache()
    trace = bool(int(os.environ.get("BASS_KERNEL_PROFILE", "0")))
    r = run_bass_kernel_spmd(_prog(name), in_maps, CORE_IDS, trace=trace)
    if trace:
        LAST_EXEC_NS.append((name, r.exec_time_ns, r.mean_exec_time_ns))
    return r.results


def _dev_shard(arr, c):
    return arr[c * CORE_N: c * CORE_N + DEV_N]


def _tail_shard(arr, c):
    return arr[c * CORE_N + DEV_N: (c + 1) * CORE_N]


def _exact_counts(x, sig, edges):
    """Host fallback for degenerate h == 0."""
    cnt = (x[:, None] <= edges[None, :]).sum(axis=0).astype(np.float64)
    sg = (x[sig][:, None] <= edges[None, :]).sum(axis=0).astype(np.float64)
    lt = (x[:, None] < edges[None, :]).sum(axis=0).astype(np.float64)
    sglt = (x[sig][:, None] < edges[None, :]).sum(axis=0).astype(np.float64)
    return cnt, sg, lt, sglt


def kernel(inputs: np.ndarray, targets: np.ndarray) -> np.ndarray:
    x_full = np.ascontiguousarray(inputs[:, 0]).astype(np.float32, copy=False)
    y_full = np.asarray(targets)
    sig_mask = y_full == 1

    tails_x = [_tail_shard(x_full, c) for c in CORE_IDS]
    tails_y = [_tail_shard(y_full, c) for c in CORE_IDS]
    tail_x = np.concatenate(tails_x)
    tail_y = np.concatenate(tails_y)

    # ---- L1: global min/max -------------------------------------------------
    LAST_EXEC_NS.clear()
    res1 = _run("minmax", [{"x": _dev_shard(x_full, c)} for c in CORE_IDS])
    gmin = np.float32(min(min(r["mm"][:, :MM_CH].min() for r in res1), tail_x.min()))
    gmax = np.float32(max(max(r["mm"][:, MM_CH:].max() for r in res1), tail_x.max()))

    # ---- edges: replicate jnp.linspace bit-exactly (eager CPU jax) ----------
    import jax
    import jax.numpy as jnp

    cpu = jax.devices("cpu")[0]
    with jax.default_device(cpu):
        edges = np.asarray(jnp.linspace(jnp.float32(gmin), jnp.float32(gmax), E))

    h = (np.float32(gmax) - np.float32(gmin)) / np.float32(N_BINS)

    if h > 0:
        inv_h = np.float32(1.0) / h
        u32 = (x_full - gmin) * inv_h
        u16 = u32.astype(np.float16)

        ned = np.ascontiguousarray(
            np.broadcast_to(-np.arange(E, dtype=np.float32), (P, E))
        )

        # per-core class compaction into dense S/B streams (SENT padded)
        in_maps = []
        n_sig_cores = []
        n_bg_cores = []
        extra_mask = np.zeros(N, bool)  # overflow elements handled exactly
        for c in CORE_IDS:
            sl = slice(c * CORE_N, c * CORE_N + DEV_N)
            uc = u16[sl]
            sg = sig_mask[sl]
            sv = uc[sg]
            bv = uc[~sg]
            if sv.size > STREAM_N:
                ovf = np.flatnonzero(sg)[STREAM_N:] + c * CORE_N
                extra_mask[ovf] = True
                sv = sv[:STREAM_N]
            if bv.size > STREAM_N:
                ovf = np.flatnonzero(~sg)[STREAM_N:] + c * CORE_N
                extra_mask[ovf] = True
                bv = bv[:STREAM_N]
            sarr = np.full(STREAM_N, SENT, np.float16)
            sarr[: sv.size] = sv
            barr = np.full(STREAM_N, SENT, np.float16)
            barr[: bv.size] = bv
            in_maps.append({"sd": sarr, "bd": barr, "ned": ned})
            n_sig_cores.append(sv.size)
            n_bg_cores.append(bv.size)

        res2 = _run("counts", in_maps)

        # aggregate device stats: D[j, r] with r=0 S (signal), r=1 B
        D_stat = np.zeros((E, 2), np.float64)
        is_sign = np.zeros((E, 2), bool)
        for r in res2:
            a = r["acc_v"].astype(np.float64).sum(axis=0)
            for i, (j, reg) in enumerate(VST):
                D_stat[j, reg] += a[i]
            D_stat[VST[0][0], VST[0][1]] += a[N_VST]  # stat 0's second half
            a = r["acc_s"].astype(np.float64).sum(axis=0)
            for i, (j, reg) in enumerate(SST):
                D_stat[j, reg] += a[i]
                is_sign[j, reg] = True
        n_sig_dev = int(np.sum(n_sig_cores))
        n_bg_dev = int(np.sum(n_bg_cores))
        sent_tot = {
            0: N_CORES * STREAM_N - n_sig_dev,
            1: N_CORES * STREAM_N - n_bg_dev,
        }
        n_real = {0: n_sig_dev, 1: n_bg_dev}

        # ---- exact corrections from near-edge candidates --------------------
        k_near = np.rint(u32)
        cand = np.abs(u32 - k_near) < np.float32(W_U)
        cidx = np.flatnonzero(cand)
        ck = np.clip(k_near[cidx].astype(np.int64), 0, E - 1)
        cx = x_full[cidx]
        cu = u16[cidx].astype(np.float32)
        csig = sig_mask[cidx]
        dev_mask = np.zeros(N, bool)
        dev_mask.reshape(N_CORES, CORE_N)[:, :DEV_N] = True
        cdev_pred = dev_mask[cidx]
        cdev = cdev_pred & ~extra_mask[cidx]

        f_exact = (cx <= edges[ck]).astype(np.float64)
        g_isle = (cu <= ck).astype(np.float64)
        s_sign = np.sign(cu - ck).astype(np.float64)

        def bc(mask, w=None):
            if w is None:
                return np.bincount(ck[mask], minlength=E).astype(np.float64)
            return np.bincount(ck[mask], weights=w[mask], minlength=E)

        le_dev = np.zeros((E, 2), np.float64)
        for reg, m_reg in ((0, cdev & csig), (1, cdev & ~csig)):
            ncand = bc(m_reg)
            F_r = bc(m_reg, f_exact)
            G_r = bc(m_reg, g_isle)
            S_r = bc(m_reg, s_sign)
            sgn = is_sign[:, reg]
            real_sign = D_stat[:, reg] - sent_tot[reg]
            le_dev[:, reg] = np.where(
                sgn,
                (n_real[reg] - ncand - (real_sign - S_r)) / 2.0 + F_r,
                D_stat[:, reg] + (F_r - G_r),
            )
            # edges 0 and 50 have no device stat: x<=gmin only for exact
            # ties (all candidates), x<=gmax holds for every element.
            le_dev[0, reg] = F_r[0]
            le_dev[E - 1, reg] = (n_real[reg] - ncand[E - 1]) + F_r[E - 1]
        sig_le = le_dev[:, 0].copy()
        cnt_le = le_dev[:, 0] + le_dev[:, 1]

        # overflow extras (ultra-rare), exact
        if extra_mask.any():
            ex = np.flatnonzero(extra_mask)
            exx = x_full[ex]
            exs = sig_mask[ex]
            cnt_le += (exx[:, None] <= edges[None, :]).sum(axis=0)
            sig_le += (exx[exs][:, None] <= edges[None, :]).sum(axis=0)

        # tails, exact
        cnt_le = cnt_le + (tail_x[:, None] <= edges[None, :]).sum(axis=0)
        sig_le = sig_le + (tail_x[tail_y == 1][:, None] <= edges[None, :]).sum(axis=0)

        # exact tie counts for lt derivation (over ALL elements; ties are
        # always candidates, including tail/extra elements)
        tie_all = (x_full[cidx] == edges[ck])
        T_all = bc(tie_all)
        Tsig_all = bc(tie_all & csig)
        cnt_lt = cnt_le - T_all
        sig_lt = sig_le - Tsig_all
    else:
        cnt_le, sig_le, cnt_lt, sig_lt = _exact_counts(x_full, sig_mask, edges)

    ns_le = sig_le.astype(np.float32)
    ns_lt = sig_lt.astype(np.float32)
    nb_le = (cnt_le - sig_le).astype(np.float32)
    nb_lt = (cnt_lt - sig_lt).astype(np.float32)

    # ---- replicate the reference's tiny pair search (eager CPU jax) ---------
    with jax.default_device(cpu):
        ns_le_j = jnp.asarray(ns_le)
        ns_lt_j = jnp.asarray(ns_lt)
        nb_le_j = jnp.asarray(nb_le)
        nb_lt_j = jnp.asarray(nb_lt)
        n_f = jnp.float32(N)
        Ns = ns_le_j[-1]
        Nb = n_f - Ns

        hist0 = nb_le_j[1:] - nb_lt_j[:-1]
        hist1 = ns_le_j[1:] - ns_lt_j[:-1]

        gt0 = hist0 > hist1
        cand0 = jnp.logical_xor(gt0[:-1], gt0[1:]) & (hist0[:-1] > 0)
        gt1 = hist1 > hist0
        cand1 = jnp.logical_xor(gt1[:-1], gt1[1:]) & (hist1[:-1] > 0)
        mask = jnp.zeros((E,), bool).at[1:N_BINS].set(cand0 | cand1)
        cnt = jnp.sum(mask)
        mask = mask.at[-1].set(mask[-1] | (cnt == 1))

        a_c = -jnp.log1p(jnp.float32(-EPS))
        b_c = -jnp.log(jnp.float32(EPS))

        def bce(correct):
            return ((n_f - correct) * b_c + correct * a_c) / n_f

        c0 = ns_le_j + (Nb - nb_le_j)
        c1 = (Ns - ns_lt_j) + nb_lt_j
        c2 = (ns_le_j[None, :] - ns_lt_j[:, None]) + Nb - (
            nb_le_j[None, :] - nb_lt_j[:, None]
        )
        c3 = ns_le_j[:, None] + (Ns - ns_lt_j[None, :]) + (
            nb_le_j[None, :] - nb_lt_j[:, None]
        )

        L = jnp.stack(
            [
                jnp.broadcast_to(bce(c0)[:, None], (E, E)),
                jnp.broadcast_to(bce(c1)[:, None], (E, E)),
                bce(c2),
                bce(c3),
            ]
        )
        per_pair_min = jnp.min(L, axis=0)
        per_pair_case = jnp.argmin(L, axis=0)

        idxs = jnp.arange(E)
        valid = mask[:, None] & mask[None, :] & (idxs[:, None] < idxs[None, :])
        flat = jnp.argmin(jnp.where(valid, per_pair_min, jnp.inf))
        i = int(flat) // E
        j = int(flat) % E
        lower = np.float32(edges[i])
        upper = np.float32(edges[j])
        case = int(per_pair_case[i, j])

    # ---- L3: predicate ------------------------------------------------------
    def exact_pred(xa):
        if case == 0:
            return xa <= lower
        if case == 1:
            return xa >= lower
        if case == 2:
            return (xa >= lower) & (xa <= upper)
        return (xa <= lower) | (xa >= upper)

    out = np.empty(N, np.int32)
    if h > 0:
        prm = np.zeros((P, 8), np.float32)
        prm[:, 0] = np.float32(i)
        prm[:, 1] = np.float32(j)
        res3 = _run(
            f"pred{case}",
            [{"u": _dev_shard(u16, c), "prm": prm} for c in CORE_IDS],
        )
        for c in CORE_IDS:
            out[c * CORE_N: c * CORE_N + DEV_N] = res3[c]["pred"].astype(np.int32)
        # overwrite candidates near the two chosen edges with the exact result
        sel = cdev_pred & ((ck == i) | (ck == j))
        sidx = cidx[sel]
        out[sidx] = exact_pred(x_full[sidx]).astype(np.int32)
    else:
        for c in CORE_IDS:
            s = slice(c * CORE_N, c * CORE_N + DEV_N)
            out[s] = exact_pred(x_full[s]).astype(np.int32)

    for c in CORE_IDS:
        out[c * CORE_N + DEV_N: (c + 1) * CORE_N] = exact_pred(tails_x[c]).astype(np.int32)
    return out


# revision 5
# speedup vs baseline: 1.0153x; 1.0153x over previous
"""Trainium2 Bass kernel for nn_CutLayer (histogram_binning) — v2.

Strategy (data-parallel over events, 8 cores):
  L1 minmax: chunked DMA of the fp32 feature column + DVE min/max reduces.
  L2 counts in u-space: host preps u16 = fp16((x-gmin)*inv_h) and a
      signal-masked copy us16 (sentinel 60000). Edges are the integers
      0..50, compile-time immediates. Per-edge cumulative counts are
      computed as fused compare+accumulate passes split across three
      engines:
        - DVE: tensor_scalar(is_le)+accum on fp16 (4x perf mode),
        - ACT: Sign activation + accum,
        - GPSIMD: tensor_scalar(is_le)+accum.
      Host corrects counts exactly using a near-edge candidate set
      (fp16 rounding can only flip compares within |u - j| < W), then
      replicates the reference's tiny E^2 pair search bit-exactly with
      eager CPU jax.
  L3 pred: case-specialized fp16 compare(s) on the u16 stream, chunked
      and pipelined; host overwrites near-edge candidates with the exact
      fp32 predicate.

Events per core: 1_000_000; device handles 128*7812 = 999_936; the
64-per-core remainder is handled exactly on the host.
"""

from contextlib import ExitStack

import numpy as np

import concourse.bass as bass
import concourse.mybir as mybir
from concourse.bass_utils import run_bass_kernel_spmd

N = 8_000_000
N_CORES = 8
CORE_N = N // N_CORES            # 1_000_000
P = 128
F = 7812                         # free-dim columns per partition
H = F // 2
Q = F // 4                       # 1953
DEV_N = P * F                    # 999_936 device events per core
N_DEV_TOT = DEV_N * N_CORES      # 7_999_488
N_BINS = 50
E = N_BINS + 1                   # 51 edges
EPS = 1e-7
SENT = np.float16(60000.0)       # sentinel > any u value (u <= ~51)
W_U = 0.05                       # candidate window in u units

# Streams are class-compacted: S = u16 of signal events (dense), B = u16 of
# background events (dense), each padded with SENT to [P, F2].
F2 = 3936                        # 128*3936 = 503_808 slots (+7.7 sigma binomial)
STREAM_N = P * F2

# stat list: (edge j, region r) with r=0 -> S, r=1 -> B; edges 0 and 50 are
# derived exactly on the host (x<=gmin ties and x<=gmax trivially), so only
# edges 1..49 need device stats: 98 total.
# Region-major so the DVE (which gets S stats) can start as soon as S lands.
# First N_VST stats go to DVE (STT+ones, is_le), rest to ACT (Sign).
DEV_EDGES = list(range(1, N_BINS))
ALL_STATS = [(j, 0) for j in DEV_EDGES] + [(j, 1) for j in DEV_EDGES]
N_VST = 46                       # DVE stats (tuned: 4.21us/stat vs ACT 3.68)
VST = ALL_STATS[:N_VST]
SST = ALL_STATS[N_VST:]
N_SST = len(SST)

FP32 = mybir.dt.float32
FP16 = mybir.dt.float16
I32 = mybir.dt.int32
AX = mybir.AxisListType
OP = mybir.AluOpType
ACT = mybir.ActivationFunctionType

CORE_IDS = list(range(N_CORES))


# --------------------------------------------------------------------------
# Bass programs
# --------------------------------------------------------------------------

MM_CH = 8                        # minmax DMA/compute chunks
MM_B = [round(F * c / MM_CH) for c in range(MM_CH + 1)]  # chunk boundaries


def _build_minmax():
    nc = bass.Bass()
    x = nc.declare_dram_parameter("x", [DEV_N], FP32, isOutput=False)
    mm = nc.declare_dram_parameter("mm", [P, 2 * MM_CH], FP32, isOutput=True)
    with ExitStack() as es:
        ec = es.enter_context
        xt = ec(nc.sbuf_tensor([P, F], FP32))
        acc = ec(nc.sbuf_tensor([P, 2 * MM_CH], FP32))
        ds = [ec(nc.semaphore(f"d{c}")) for c in range(MM_CH)]
        vsem = ec(nc.semaphore("vsem"))
        dso = ec(nc.semaphore("dso"))
        block = ec(nc.Block())

        @block.sync
        def _(sync):
            xv = x[:].rearrange("(p f) -> p f", p=P)
            for c in range(MM_CH):
                sync.dma_start(
                    xt[:, MM_B[c]:MM_B[c+1]], xv[:, MM_B[c]:MM_B[c+1]]
                ).then_inc(ds[c], 16)
            sync.wait_ge(vsem, 2 * MM_CH)
            sync.dma_start(mm[:], acc[:]).then_inc(dso, 16)
            sync.wait_ge(dso, 16)

        @block.vector
        def _(vector):
            for c in range(MM_CH):
                vector.wait_ge(ds[c], 16)
                sl = xt[:, MM_B[c]:MM_B[c+1]]
                vector.tensor_reduce(acc[:, c:c+1], sl, axis=AX.X, op=OP.min).then_inc(vsem, 1)
                vector.tensor_reduce(acc[:, MM_CH+c:MM_CH+c+1], sl, axis=AX.X, op=OP.max).then_inc(vsem, 1)
    return nc


def _build_counts():
    nc = bass.Bass()
    sd = nc.declare_dram_parameter("sd", [STREAM_N], FP16, isOutput=False)
    bd = nc.declare_dram_parameter("bd", [STREAM_N], FP16, isOutput=False)
    ned = nc.declare_dram_parameter("ned", [P, E], FP32, isOutput=False)
    # one extra accumulator column: stat 0 runs as two half-stream instrs
    # (cols 0 and N_VST) so the DVE starts before the S stream fully lands
    av = nc.declare_dram_parameter("acc_v", [P, N_VST + 1], FP32, isOutput=True)
    asn = nc.declare_dram_parameter("acc_s", [P, N_SST], FP32, isOutput=True)
    H2 = F2 // 2
    with ExitStack() as es:
        ec = es.enter_context
        st = ec(nc.sbuf_tensor([P, F2], FP16))
        bt = ec(nc.sbuf_tensor([P, F2], FP16))
        ones = ec(nc.sbuf_tensor([P, F2], FP16))
        scrv = ec(nc.sbuf_tensor([P, F2], FP16))
        scrs = ec(nc.sbuf_tensor([P, F2], FP16))
        edt = ec(nc.sbuf_tensor([P, E], FP32))
        atv = ec(nc.sbuf_tensor([P, N_VST + 1], FP32))
        ats = ec(nc.sbuf_tensor([P, N_SST], FP32))
        dss0 = ec(nc.semaphore("dss0"))
        dss = ec(nc.semaphore("dss"))
        dsb = ec(nc.semaphore("dsb"))
        dne = ec(nc.semaphore("dne"))
        vsem = ec(nc.semaphore("vsem"))
        ssem = ec(nc.semaphore("ssem"))
        dso = ec(nc.semaphore("dso"))
        block = ec(nc.Block())

        @block.sync
        def _(sync):
            sv = sd[:].rearrange("(p f) -> p f", p=P)
            bv = bd[:].rearrange("(p f) -> p f", p=P)
            sync.dma_start(edt[:], ned[:]).then_inc(dne, 16)
            sync.dma_start(st[:, 0:H2], sv[:, 0:H2]).then_inc(dss0, 16)
            sync.dma_start(st[:, H2:F2], sv[:, H2:F2]).then_inc(dss, 16)
            sync.dma_start(bt[:], bv).then_inc(dsb, 16)
            sync.wait_ge(ssem, N_SST)
            sync.dma_start(asn[:], ats[:]).then_inc(dso, 16)
            sync.wait_ge(vsem, N_VST + 1)
            sync.dma_start(av[:], atv[:]).then_inc(dso, 16)
            sync.wait_ge(dso, 32)

        # DVE: STT is_le * ones with fp32 accum; stats in VST order (S first).
        # Stat 0 runs as two half-stream instrs so compute starts once the
        # first half of S has landed.
        @block.vector
        def _(vector):
            vector.memset(ones[:], 1.0)
            j0, _r0 = VST[0]
            vector.wait_ge(dss0, 16)
            vector.scalar_tensor_tensor(
                scrv[:, 0:H2], st[:, 0:H2], float(j0), ones[:, 0:H2],
                op0=OP.is_le, op1=OP.mult,
                accum_out=atv[:, 0:1],
            ).then_inc(vsem, 1)
            vector.wait_ge(dss, 16)
            vector.scalar_tensor_tensor(
                scrv[:, 0:H2], st[:, H2:F2], float(j0), ones[:, 0:H2],
                op0=OP.is_le, op1=OP.mult,
                accum_out=atv[:, N_VST:N_VST + 1],
            ).then_inc(vsem, 1)
            waited = {0}
            for i, (j, r) in enumerate(VST):
                if i == 0:
                    continue
                sem = dss if r == 0 else dsb
                if r not in waited:
                    waited.add(r)
                    vector.wait_ge(sem, 16)
                src = st if r == 0 else bt
                vector.scalar_tensor_tensor(
                    scrv[:], src[:], float(j), ones[:],
                    op0=OP.is_le, op1=OP.mult,
                    accum_out=atv[:, i:i + 1],
                ).then_inc(vsem, 1)

        # ACT: Sign with fp32 accum; bias = -j from the edges tile.
        @block.scalar
        def _(scalar):
            scalar.wait_ge(dne, 16)
            # dummy pass on the (tiny, already-loaded) edges tile: pulls the
            # Sign ACT table load off the critical path while streams land
            scalar.activation(ats[:, 0:1], edt[:, 0:1], ACT.Sign,
                              bias=edt[:, 0:1], scale=1.0)
            waited = set()
            for i, (j, r) in enumerate(SST):
                if r not in waited:
                    waited.add(r)
                    if r == 0:
                        scalar.wait_ge(dss0, 16)
                        scalar.wait_ge(dss, 16)
                    else:
                        scalar.wait_ge(dsb, 16)
                src = st if r == 0 else bt
                scalar.activation(
                    scrs[:], src[:], ACT.Sign,
                    bias=edt[:, j:j + 1], scale=1.0,
                    accum_out=ats[:, i:i + 1],
                ).then_inc(ssem, 1)
    return nc


PR_CH = 4                        # pred DMA/compute chunks
PR_B = [round(F * c / PR_CH) for c in range(PR_CH + 1)]
PR_Q = max(PR_B[c+1] - PR_B[c] for c in range(PR_CH))


def _build_pred(case: int):
    """u-space predicate, chunked.
    0: u <= lo ; 1: u >= lo ; 2: (u >= lo) & (u <= up) ; 3: (u <= lo) + (u >= up)
    """
    nc = bass.Bass()
    u = nc.declare_dram_parameter("u", [DEV_N], FP16, isOutput=False)
    pr = nc.declare_dram_parameter("prm", [P, 8], FP32, isOutput=False)
    out = nc.declare_dram_parameter("pred", [DEV_N], FP16, isOutput=True)
    with ExitStack() as es:
        ec = es.enter_context
        ut = ec(nc.sbuf_tensor([P, F], FP16))
        po = ec(nc.sbuf_tensor([P, F], FP16))
        t1 = ec(nc.sbuf_tensor([P, PR_Q], FP16))
        t2 = ec(nc.sbuf_tensor([P, PR_Q], FP16))
        prm = ec(nc.sbuf_tensor([P, 8], FP32))
        dp = ec(nc.semaphore("dp"))
        ds = [ec(nc.semaphore(f"d{c}")) for c in range(PR_CH)]
        vsem = ec(nc.semaphore("vsem"))
        dso = ec(nc.semaphore("dso"))
        block = ec(nc.Block())

        @block.sync
        def _(sync):
            uv = u[:].rearrange("(p f) -> p f", p=P)
            ov = out[:].rearrange("(p f) -> p f", p=P)
            sync.dma_start(prm[:], pr[:]).then_inc(dp, 16)
            for c in range(PR_CH):
                sync.dma_start(
                    ut[:, PR_B[c]:PR_B[c+1]], uv[:, PR_B[c]:PR_B[c+1]]
                ).then_inc(ds[c], 16)
            for c in range(PR_CH):
                sync.wait_ge(vsem, c + 1)
                sync.dma_start(
                    ov[:, PR_B[c]:PR_B[c+1]], po[:, PR_B[c]:PR_B[c+1]]
                ).then_inc(dso, 16)
            sync.wait_ge(dso, 16 * PR_CH)

        @block.vector
        def _(vector):
            vector.wait_ge(dp, 16)
            lo = prm[:, 0:1]
            up = prm[:, 1:2]
            for c in range(PR_CH):
                vector.wait_ge(ds[c], 16)
                w = PR_B[c+1] - PR_B[c]
                uc = ut[:, PR_B[c]:PR_B[c+1]]
                oc = po[:, PR_B[c]:PR_B[c+1]]
                if case == 0:
                    vector.tensor_scalar(oc, uc, lo, 1.0, OP.is_le, OP.mult).then_inc(vsem, 1)
                elif case == 1:
                    vector.tensor_scalar(oc, uc, lo, 1.0, OP.is_ge, OP.mult).then_inc(vsem, 1)
                elif case == 2:
                    vector.tensor_scalar(t1[:, 0:w], uc, lo, 1.0, OP.is_ge, OP.mult)
                    vector.tensor_scalar(t2[:, 0:w], uc, up, 1.0, OP.is_le, OP.mult)
                    vector.tensor_tensor(oc, t1[:, 0:w], t2[:, 0:w], op=OP.mult).then_inc(vsem, 1)
                else:
                    vector.tensor_scalar(t1[:, 0:w], uc, lo, 1.0, OP.is_le, OP.mult)
                    vector.tensor_scalar(t2[:, 0:w], uc, up, 1.0, OP.is_ge, OP.mult)
                    vector.tensor_tensor(oc, t1[:, 0:w], t2[:, 0:w], op=OP.add).then_inc(vsem, 1)
    return nc


_PROGRAMS: dict = {}


def _prog(name):
    if name not in _PROGRAMS:
        if name.startswith("pred"):
            _PROGRAMS[name] = _build_pred(int(name[4:]))
        else:
            _PROGRAMS[name] = {
                "minmax": _build_minmax,
                "counts": _build_counts,
            }[name]()
    return _PROGRAMS[name]


# --------------------------------------------------------------------------
# Host orchestration
# --------------------------------------------------------------------------

LAST_EXEC_NS: list = []
_CACHE_SET = False


def _enable_jit_cache():
    global _CACHE_SET
    if _CACHE_SET:
        return
    _CACHE_SET = True
    try:
        import jax

        jax.config.update("jax_compilation_cache_dir", "/tmp/jax_bass_cache")
        jax.config.update("jax_persistent_cache_min_compile_time_secs", 1.0)
        jax.config.update("jax_persistent_cache_min_entry_size_bytes", 0)
    except Exception:
        pass


def _run(name, in_maps):
    import os

    _enable_jit_cache()
    trace = bool(int(os.environ.get("BASS_KERNEL_PROFILE", "0")))
    r = run_bass_kernel_spmd(_prog(name), in_maps, CORE_IDS, trace=trace)
    if trace:
        LAST_EXEC_NS.append((name, r.exec_time_ns, r.mean_exec_time_ns))
    return r.results


def _dev_shard(arr, c):
    return arr[c * CORE_N: c * CORE_N + DEV_N]


def _tail_shard(arr, c):
    return arr[c * CORE_N + DEV_N: (c + 1) * CORE_N]


def _exact_counts(x, sig, edges):
    """Host fallback for degenerate h == 0."""
    cnt = (x[:, None] <= edges[None, :]).sum(axis=0).astype(np.float64)
    sg = (x[sig][:, None] <= edges[None, :]).sum(axis=0).astype(np.float64)
    lt = (x[:, None] < edges[None, :]).sum(axis=0).astype(np.float64)
    sglt = (x[sig][:, None] < edges[None, :]).sum(axis=0).astype(np.float64)
    return cnt, sg, lt, sglt


def kernel(inputs: np.ndarray, targets: np.ndarray) -> np.ndarray:
    x_full = np.ascontiguousarray(inputs[:, 0]).astype(np.float32, copy=False)
    y_full = np.asarray(targets)
    sig_mask = y_full == 1

    tails_x = [_tail_shard(x_full, c) for c in CORE_IDS]
    tails_y = [_tail_shard(y_full, c) for c in CORE_IDS]
    tail_x = np.concatenate(tails_x)
    tail_y = np.concatenate(tails_y)

    # ---- L1: global min/max -------------------------------------------------
    LAST_EXEC_NS.clear()
    res1 = _run("minmax", [{"x": _dev_shard(x_full, c)} for c in CORE_IDS])
    gmin = np.float32(min(min(r["mm"][:, :MM_CH].min() for r in res1), tail_x.min()))
    gmax = np.float32(max(max(r["mm"][:, MM_CH:].max() for r in res1), tail_x.max()))

    # ---- edges: replicate jnp.linspace bit-exactly (eager CPU jax) ----------
    import jax
    import jax.numpy as jnp

    cpu = jax.devices("cpu")[0]
    with jax.default_device(cpu):
        edges = np.asarray(jnp.linspace(jnp.float32(gmin), jnp.float32(gmax), E))

    h = (np.float32(gmax) - np.float32(gmin)) / np.float32(N_BINS)

    if h > 0:
        inv_h = np.float32(1.0) / h
        u32 = (x_full - gmin) * inv_h
        u16 = u32.astype(np.float16)

        ned = np.ascontiguousarray(
            np.broadcast_to(-np.arange(E, dtype=np.float32), (P, E))
        )

        # per-core class compaction into dense S/B streams (SENT padded)
        in_maps = []
        n_sig_cores = []
        n_bg_cores = []
        extra_mask = np.zeros(N, bool)  # overflow elements handled exactly
        for c in CORE_IDS:
            sl = slice(c * CORE_N, c * CORE_N + DEV_N)
            uc = u16[sl]
            sg = sig_mask[sl]
            sv = uc[sg]
            bv = uc[~sg]
            if sv.size > STREAM_N:
                ovf = np.flatnonzero(sg)[STREAM_N:] + c * CORE_N
                extra_mask[ovf] = True
                sv = sv[:STREAM_N]
            if bv.size > STREAM_N:
                ovf = np.flatnonzero(~sg)[STREAM_N:] + c * CORE_N
                extra_mask[ovf] = True
                bv = bv[:STREAM_N]
            sarr = np.full(STREAM_N, SENT, np.float16)
            sarr[: sv.size] = sv
            barr = np.full(STREAM_N, SENT, np.float16)
            barr[: bv.size] = bv
            in_maps.append({"sd": sarr, "bd": barr, "ned": ned})
            n_sig_cores.append(sv.size)
            n_bg_cores.append(bv.size)

        res2 = _run("counts", in_maps)

        # aggregate device stats: D[j, r] with r=0 S (signal), r=1 B
        D_stat = np.zeros((E, 2), np.float64)
        is_sign = np.zeros((E, 2), bool)
        for r in res2:
            a = r["acc_v"].astype(np.float64).sum(axis=0)
            for i, (j, reg) in enumerate(VST):
                D_stat[j, reg] += a[i]
            D_stat[VST[0][0], VST[0][1]] += a[N_VST]  # stat 0's second half
            a = r["acc_s"].astype(np.float64).sum(axis=0)
            for i, (j, reg) in enumerate(SST):
                D_stat[j, reg] += a[i]
                is_sign[j, reg] = True
        n_sig_dev = int(np.sum(n_sig_cores))
        n_bg_dev = int(np.sum(n_bg_cores))
        sent_tot = {
            0: N_CORES * STREAM_N - n_sig_dev,
            1: N_CORES * STREAM_N - n_bg_dev,
        }
        n_real = {0: n_sig_dev, 1: n_bg_dev}

        # ---- exact corrections from near-edge candidates --------------------
        k_near = np.rint(u32)
        cand = np.abs(u32 - k_near) < np.float32(W_U)
        cidx = np.flatnonzero(cand)
        ck = np.clip(k_near[cidx].astype(np.int64), 0, E - 1)
        cx = x_full[cidx]
        cu = u16[cidx].astype(np.float32)
        csig = sig_mask[cidx]
        dev_mask = np.zeros(N, bool)
        dev_mask.reshape(N_CORES, CORE_N)[:, :DEV_N] = True
        cdev_pred = dev_mask[cidx]
        cdev = cdev_pred & ~extra_mask[cidx]

        f_exact = (cx <= edges[ck]).astype(np.float64)
        g_isle = (cu <= ck).astype(np.float64)
        s_sign = np.sign(cu - ck).astype(np.float64)

        def bc(mask, w=None):
            if w is None:
                return np.bincount(ck[mask], minlength=E).astype(np.float64)
            return np.bincount(ck[mask], weights=w[mask], minlength=E)

        le_dev = np.zeros((E, 2), np.float64)
        for reg, m_reg in ((0, cdev & csig), (1, cdev & ~csig)):
            ncand = bc(m_reg)
            F_r = bc(m_reg, f_exact)
            G_r = bc(m_reg, g_isle)
            S_r = bc(m_reg, s_sign)
            sgn = is_sign[:, reg]
            real_sign = D_stat[:, reg] - sent_tot[reg]
            le_dev[:, reg] = np.where(
                sgn,
                (n_real[reg] - ncand - (real_sign - S_r)) / 2.0 + F_r,
                D_stat[:, reg] + (F_r - G_r),
            )
            # edges 0 and 50 have no device stat: x<=gmin only for exact
            # ties (all candidates), x<=gmax holds for every element.
            le_dev[0, reg] = F_r[0]
            le_dev[E - 1, reg] = (n_real[reg] - ncand[E - 1]) + F_r[E - 1]
        sig_le = le_dev[:, 0].copy()
        cnt_le = le_dev[:, 0] + le_dev[:, 1]

        # overflow extras (ultra-rare), exact
        if extra_mask.any():
            ex = np.flatnonzero(extra_mask)
            exx = x_full[ex]
            exs = sig_mask[ex]
            cnt_le += (exx[:, None] <= edges[None, :]).sum(axis=0)
            sig_le += (exx[exs][:, None] <= edges[None, :]).sum(axis=0)

        # tails, exact
        cnt_le = cnt_le + (tail_x[:, None] <= edges[None, :]).sum(axis=0)
        sig_le = sig_le + (tail_x[tail_y == 1][:, None] <= edges[None, :]).sum(axis=0)

        # exact tie counts for lt derivation (over ALL elements; ties are
        # always candidates, including tail/extra elements)
        tie_all = (x_full[cidx] == edges[ck])
        T_all = bc(tie_all)
        Tsig_all = bc(tie_all & csig)
        cnt_lt = cnt_le - T_all
        sig_lt = sig_le - Tsig_all
    else:
        cnt_le, sig_le, cnt_lt, sig_lt = _exact_counts(x_full, sig_mask, edges)

    ns_le = sig_le.astype(np.float32)
    ns_lt = sig_lt.astype(np.float32)
    nb_le = (cnt_le - sig_le).astype(np.float32)
    nb_lt = (cnt_lt - sig_lt).astype(np.float32)

    # ---- replicate the reference's tiny pair search (eager CPU jax) ---------
    with jax.default_device(cpu):
        ns_le_j = jnp.asarray(ns_le)
        ns_lt_j = jnp.asarray(ns_lt)
        nb_le_j = jnp.asarray(nb_le)
        nb_lt_j = jnp.asarray(nb_lt)
        n_f = jnp.float32(N)
        Ns = ns_le_j[-1]
        Nb = n_f - Ns

        hist0 = nb_le_j[1:] - nb_lt_j[:-1]
        hist1 = ns_le_j[1:] - ns_lt_j[:-1]

        gt0 = hist0 > hist1
        cand0 = jnp.logical_xor(gt0[:-1], gt0[1:]) & (hist0[:-1] > 0)
        gt1 = hist1 > hist0
        cand1 = jnp.logical_xor(gt1[:-1], gt1[1:]) & (hist1[:-1] > 0)
        mask = jnp.zeros((E,), bool).at[1:N_BINS].set(cand0 | cand1)
        cnt = jnp.sum(mask)
        mask = mask.at[-1].set(mask[-1] | (cnt == 1))

        a_c = -jnp.log1p(jnp.float32(-EPS))
        b_c = -jnp.log(jnp.float32(EPS))

        def bce(correct):
            return ((n_f - correct) * b_c + correct * a_c) / n_f

        c0 = ns_le_j + (Nb - nb_le_j)
        c1 = (Ns - ns_lt_j) + nb_lt_j
        c2 = (ns_le_j[None, :] - ns_lt_j[:, None]) + Nb - (
            nb_le_j[None, :] - nb_lt_j[:, None]
        )
        c3 = ns_le_j[:, None] + (Ns - ns_lt_j[None, :]) + (
            nb_le_j[None, :] - nb_lt_j[:, None]
        )

        L = jnp.stack(
            [
                jnp.broadcast_to(bce(c0)[:, None], (E, E)),
                jnp.broadcast_to(bce(c1)[:, None], (E, E)),
                bce(c2),
                bce(c3),
            ]
        )
        per_pair_min = jnp.min(L, axis=0)
        per_pair_case = jnp.argmin(L, axis=0)

        idxs = jnp.arange(E)
        valid = mask[:, None] & mask[None, :] & (idxs[:, None] < idxs[None, :])
        flat = jnp.argmin(jnp.where(valid, per_pair_min, jnp.inf))
        i = int(flat) // E
        j = int(flat) % E
        lower = np.float32(edges[i])
        upper = np.float32(edges[j])
        case = int(per_pair_case[i, j])

    # ---- L3: predicate ------------------------------------------------------
    def exact_pred(xa):
        if case == 0:
            return xa <= lower
        if case == 1:
            return xa >= lower
        if case == 2:
            return (xa >= lower) & (xa <= upper)
        return (xa <= lower) | (xa >= upper)

    out = np.empty(N, np.int32)
    if h > 0:
        prm = np.zeros((P, 8), np.float32)
        prm[:, 0] = np.float32(i)
        prm[:, 1] = np.float32(j)
        res3 = _run(
            f"pred{case}",
            [{"u": _dev_shard(u16, c), "prm": prm} for c in CORE_IDS],
        )
        for c in CORE_IDS:
            out[c * CORE_N: c * CORE_N + DEV_N] = res3[c]["pred"].astype(np.int32)
        # overwrite candidates near the two chosen edges with the exact result
        sel = cdev_pred & ((ck == i) | (ck == j))
        sidx = cidx[sel]
        out[sidx] = exact_pred(x_full[sidx]).astype(np.int32)
    else:
        for c in CORE_IDS:
            s = slice(c * CORE_N, c * CORE_N + DEV_N)
            out[s] = exact_pred(x_full[s]).astype(np.int32)

    for c in CORE_IDS:
        out[c * CORE_N + DEV_N: (c + 1) * CORE_N] = exact_pred(tails_x[c]).astype(np.int32)
    return out
